# revision 1
# baseline (speedup 1.0000x reference)
"""Multi-head causal attention (B=2, N=2048, D=1024, H=16) on 8 Trainium2 cores.

Sharding: tensor-parallel over heads (2 heads/core) for QKV projections and
attention; one AllToAll redistributes attention outputs to a token-sharded
layout; each core then runs the full output projection for its 512 tokens.

Layout trick: everything in attention is computed transposed ("head-dim major"):
  qT/kT/vT = [head_dim(2 heads stacked), tokens]  (from W @ x^T matmuls)
  scores^T = K Q^T per (batch, head)              (PE row-packed: 2 heads concurrent)
  P^T = exp(scores^T)                             (causal-masked via affine_select /
                                                   0/1 block masks)
  attn^T  = (V1^T P)                              (V1 has a ones column -> softmax
                                                   denominator rides along for free)
No softmax max-subtraction (scores are O(1) here; exp can't overflow), so the
softmax is exp -> matmul-accumulated row sums -> fast-reciprocal scaling.

The QKV projection chunks are emitted interleaved with the attention
(batch, q-half) sections they feed, sharing one PSUM pool, so the Tile
scheduler can overlap projection matmuls, exp, and attention-value matmuls.

Matmuls run in float32r (full PE rate, ~1e-4 relative error).
"""

import numpy as np

from concourse import bacc, tile, mybir
import concourse.bass as bass
from concourse.bass_utils import run_bass_kernel_spmd

NCORES = 8
B, N, D, H, HD = 2, 2048, 1024, 16, 64
TOK = B * N              # 4096
HPC = H // NCORES        # 2 heads per core
TPC = TOK // NCORES      # 512 output tokens per core
BT = 128                 # attention block size
NB = N // BT             # 16 key blocks per batch
QHW = 1024               # q-half width for PSUM-resident AV accumulation
CH = 512                 # token chunk for QKV projection matmuls
F32 = mybir.dt.float32
F32R = mybir.dt.float32r
EXP = mybir.ActivationFunctionType.Exp


def _mm(ap):
    return ap


def make_plan(mask):
    """Analyze the [1,1,N,N] mask into per-key-block structure."""
    m = np.asarray(mask).reshape(N, N)
    runs = {}
    partial = {}
    patterns = []
    pat_keys = {}
    nchunks = N // 512
    first_kb = [None] * nchunks
    last_kb = [None] * nchunks
    for kb in range(NB):
        valid_qbs = []
        for qb in range(NB):
            blk = m[qb * BT:(qb + 1) * BT, kb * BT:(kb + 1) * BT]
            if not blk.any():
                continue
            valid_qbs.append(qb)
            if not blk.all():
                pat = np.ascontiguousarray(blk.T.astype(np.float32))
                key = pat.tobytes()
                if key not in pat_keys:
                    pat_keys[key] = len(patterns)
                    patterns.append(pat)
                partial.setdefault(kb, []).append((qb, pat_keys[key]))
        rr = []
        for qb in valid_qbs:
            if rr and rr[-1][1] == qb * BT:
                rr[-1][1] = (qb + 1) * BT
            else:
                rr.append([qb * BT, (qb + 1) * BT])
        runs[kb] = [tuple(r) for r in rr]
        for (c0, c1) in runs[kb]:
            for cc in range(c0 // 512, (c1 - 1) // 512 + 1):
                if first_kb[cc] is None:
                    first_kb[cc] = kb
                last_kb[cc] = kb
    if not patterns:
        patterns.append(np.ones((BT, BT), np.float32))
    causal_pat = (np.arange(BT)[None, :] >= np.arange(BT)[:, None]).astype(np.float32)
    is_causal = [bool((p == causal_pat).all()) for p in patterns]
    return {
        "is_causal": is_causal,
        "runs": runs,
        "partial": partial,
        "patterns": np.stack(patterns),
        "first_kb": first_kb,
        "last_kb": last_kb,
    }


def _pieces(c0, c1):
    """Split [c0, c1) at 512 boundaries (PSUM bank granularity)."""
    out = []
    c = c0
    while c < c1:
        e = min((c // 512 + 1) * 512, c1)
        out.append((c, e))
        c = e
    return out


def build_nc(plan):
    nc = bacc.Bacc("TRN2", target_bir_lowering=False, debug=False,
                   num_devices=NCORES)
    n_pat = plan["patterns"].shape[0]

    xT = nc.dram_tensor("xT", [D, TOK], F32R, kind="ExternalInput")
    wqT = nc.dram_tensor("wqT", [D, BT], F32R, kind="ExternalInput")
    wkT = nc.dram_tensor("wkT", [D, BT], F32R, kind="ExternalInput")
    wvT = nc.dram_tensor("wvT", [D, BT], F32R, kind="ExternalInput")
    woT = nc.dram_tensor("woT", [D, D], F32R, kind="ExternalInput")
    bo1 = nc.dram_tensor("bo1", [1, D], F32, kind="ExternalInput")
    ident = nc.dram_tensor("ident", [BT, BT], F32, kind="ExternalInput")
    pm = nc.dram_tensor("pm", [n_pat, BT, BT], F32, kind="ExternalInput")
    y = nc.dram_tensor("y", [TPC, D], F32, kind="ExternalOutput")

    runs, partial = plan["runs"], plan["partial"]
    first_kb, last_kb = plan["first_kb"], plan["last_kb"]
    assert len(plan["is_causal"]) == n_pat

    with tile.TileContext(nc) as tc:
        with (
            tc.tile_pool(name="const", bufs=1) as cp,
            tc.tile_pool(name="big", bufs=1) as bigp,
            tc.tile_pool(name="psum", bufs=1, space="PSUM") as psum,
            tc.tile_pool(name="dram", bufs=1, space="DRAM") as dram,
        ):
            # ---- constants ----
            identt = cp.tile([BT, BT], F32, name="identt")
            nc.sync.dma_start(identt[:], ident.ap())
            pmt = [cp.tile([BT, BT], F32, name=f"pmt{i}") for i in range(n_pat)]
            for i in range(n_pat):
                nc.sync.dma_start(pmt[i][:], pm.ap()[i])
            bot = cp.tile([1, D], F32, name="bot")
            nc.sync.dma_start(bot[:], bo1.ap())
            bobc = cp.tile([BT, D], F32, name="bobc")
            nc.gpsimd.partition_broadcast(bobc[:], bot[:])
            onecol = cp.tile([BT, 1], F32, name="onecol")
            nc.vector.memset(onecol[:], 1.0)

            # ---- warm-up collective (absorbs launch skew / collective setup) ----
            wa_in = dram.tile([BT, 4], F32, name="wa_in")
            wa_out = dram.tile([BT * NCORES, 4], F32, name="wa_out",
                               addr_space="Shared")
            nc.sync.dma_start(wa_in[:], identt[:, 0:4])
            nc.gpsimd.collective_compute(
                "AllGather", mybir.AluOpType.bypass,
                ins=[wa_in.opt()], outs=[wa_out.opt()],
                replica_groups=[list(range(NCORES))])

            a2a_in = dram.tile([NCORES * BT, TPC], F32R, name="a2a_in")
            a2a_out = dram.tile([NCORES * BT, TPC], F32R, name="a2a_out")

            qTt = bigp.tile([BT, TOK], F32R, name="qTt")
            kTt = bigp.tile([BT, TOK], F32R, name="kTt")
            v1 = [bigp.tile([BT, 132], F32R, name=f"v1_{tb}")
                  for tb in range(TOK // BT)]
            attnT = {}
            for b in range(B):
                for j in range(HPC):
                    attnT[(b, j)] = bigp.tile([HD, N], F32R, name=f"attnT{b}{j}")

            with (
                tc.tile_pool(name="wqkv", bufs=1) as wp,
                tc.tile_pool(name="xp", bufs=2) as xp,
                tc.tile_pool(name="vtp", bufs=1) as vtp,
                tc.tile_pool(name="ptp", bufs=2) as ptp,
                tc.tile_pool(name="workp", bufs=2) as workp,
            ):
                wq = [wp.tile([BT, BT], F32R, name=f"wq{e}") for e in range(8)]
                wk = [wp.tile([BT, BT], F32R, name=f"wk{e}") for e in range(8)]
                wv = [wp.tile([BT, BT], F32R, name=f"wv{e}") for e in range(8)]
                for e in range(8):
                    nc.scalar.dma_start(wq[e][:], wqT.ap()[e * BT:(e + 1) * BT, :])
                    nc.scalar.dma_start(wk[e][:], wkT.ap()[e * BT:(e + 1) * BT, :])
                    nc.scalar.dma_start(wv[e][:], wvT.ap()[e * BT:(e + 1) * BT, :])
                vTt = vtp.tile([BT, TOK], F32, name="vTt")

                def emit_qkv_chunk(ch):
                    xt = [xp.tile([BT, CH], F32R, name=f"xt{e}", tag=f"xt{e}")
                          for e in range(8)]
                    for e in range(8):
                        nc.sync.dma_start(
                            xt[e][:],
                            xT.ap()[e * BT:(e + 1) * BT, ch * CH:(ch + 1) * CH])
                    for (wt, dst, eng) in ((wq, qTt, "act"), (wk, kTt, "dve"),
                                           (wv, vTt, "dve")):
                        ps = psum.tile([BT, CH], F32, name="psqkv", tag="psqkv",
                                       bufs=2)
                        for e in range(8):
                            nc.tensor.matmul(ps[:], _mm(wt[e][:]), _mm(xt[e][:]),
                                             start=(e == 0), stop=(e == 7))
                        dslc = dst[:, ch * CH:(ch + 1) * CH]
                        if eng == "act":
                            nc.scalar.copy(dslc, ps[:])
                        else:
                            nc.vector.tensor_copy(dslc, ps[:])
                    # V1 for this chunk's token blocks (PE transpose):
                    # v1: [A(0:64) | onesA(64) | pad | B(66:130) | onesB(130)]
                    for tb in range(ch * CH // BT, (ch + 1) * CH // BT):
                        pst = psum.tile([BT, BT], F32, name="pst", tag="psqkv",
                                        bufs=2)
                        nc.tensor.transpose(pst[:],
                                            vTt[:, tb * BT:(tb + 1) * BT],
                                            identt[:])
                        dst = v1[tb][:].rearrange("p (g c) -> p g c",
                                                  g=2)[:, :, 0:64]
                        src = pst[:].rearrange("p (g c) -> p g c", g=2)
                        nc.vector.tensor_copy(dst, src)
                        nc.vector.tensor_copy(v1[tb][:, 64:65], onecol[:])
                        nc.vector.tensor_copy(v1[tb][:, 130:131], onecol[:])

                # interleave: QKV chunks for each (batch, q-half), then that
                # section's attention — one shared PSUM pool, no phase barrier
                for step in range(2 * B):
                    b, qh = divmod(step, 2)
                    emit_qkv_chunk(2 * step)
                    emit_qkv_chunk(2 * step + 1)

                    q0, q1 = qh * QHW, (qh + 1) * QHW
                    psO = [psum.tile([65, QHW], F32, name=f"psO{j}",
                                     tag=f"psO{j}") for j in range(HPC)]
                    for kb in range(NB):
                        pieces = []
                        for (c0, c1) in runs.get(kb, []):
                            lo, hi = max(c0, q0), min(c1, q1)
                            if lo < hi:
                                pieces += _pieces(lo, hi)
                        if not pieces:
                            continue
                        pT = [ptp.tile([BT, QHW], F32R, name=f"pT{j}",
                                       tag=f"pT{j}") for j in range(HPC)]
                        for j in range(HPC):
                            kslc = slice(HD * j, HD * (j + 1))
                            kcols = slice(b * N + kb * BT, b * N + (kb + 1) * BT)
                            for (c0, c1) in pieces:
                                psS = psum.tile([BT, 512], F32, name="psS",
                                                tag="psS", bufs=2)
                                w = c1 - c0
                                nc.tensor.matmul(
                                    psS[:, 0:w],
                                    _mm(kTt[kslc, kcols]),
                                    _mm(qTt[kslc, b * N + c0:b * N + c1]),
                                    start=True, stop=True,
                                    tile_position=(HD * j, 0))
                                nc.scalar.activation(
                                    pT[j][:, c0 - q0:c1 - q0], psS[:, 0:w],
                                    EXP, scale=0.125)
                            # causal / partial-block masking
                            for (qb, pidx) in partial.get(kb, []):
                                qc = qb * BT
                                if q0 <= qc < q1:
                                    slc = pT[j][:, qc - q0:qc - q0 + BT]
                                    if plan["is_causal"][pidx]:
                                        nc.gpsimd.affine_select(
                                            slc, slc, pattern=[[1, BT]],
                                            compare_op=mybir.AluOpType.is_ge,
                                            fill=0.0, base=0,
                                            channel_multiplier=-1)
                                    else:
                                        nc.vector.tensor_mul(slc, slc,
                                                             pmt[pidx][:])
                            # AV accumulate (+ denominator via ones column)
                            v1t = v1[b * NB + kb]
                            for (c0, c1) in pieces:
                                cc = c0 // 512
                                nc.tensor.matmul(
                                    psO[j][:, c0 - q0:c1 - q0],
                                    _mm(v1t[:, 66 * j:66 * j + 65]),
                                    _mm(pT[j][:, c0 - q0:c1 - q0]),
                                    start=(kb == first_kb[cc]),
                                    stop=(kb == last_kb[cc]))
                    # normalize: evacuate PSUM fast, then scale by 1/denominator
                    for j in range(HPC):
                        nc.vector.tensor_copy(attnT[(b, j)][:, q0:q1],
                                              psO[j][0:64, :])
                        dn = workp.tile([1, QHW], F32, name="dn", tag="dn")
                        nc.vector.tensor_copy(dn[:], psO[j][64:65, :])
                        recip = workp.tile([1, QHW], F32, name="recip",
                                           tag="recip")
                        nc.vector.reciprocal_approx_fast(recip[:], dn[:])
                        rbc = workp.tile([HD, QHW], F32, name="rbc", tag="rbc")
                        nc.gpsimd.partition_broadcast(rbc[:], recip[:])
                        nc.vector.tensor_mul(attnT[(b, j)][:, q0:q1],
                                             attnT[(b, j)][:, q0:q1],
                                             rbc[:])
                    if qh == 1:
                        # stage this batch's slices for the AllToAll
                        for r in range(4 * b, 4 * b + 4):
                            lc = (r % 4) * TPC
                            for j in range(HPC):
                                nc.sync.dma_start(
                                    a2a_in[BT * r + HD * j:
                                           BT * r + HD * (j + 1), :],
                                    attnT[(b, j)][:, lc:lc + TPC])

            nc.gpsimd.collective_compute(
                "AllToAll", mybir.AluOpType.bypass,
                ins=[a2a_in.opt()], outs=[a2a_out.opt()],
                replica_groups=[list(range(NCORES))])

            # ---- output projection for this core's 512 tokens ----
            with tc.tile_pool(name="fp", bufs=1) as fp:
                wo = [fp.tile([BT, D], F32R, name=f"wo{e}") for e in range(8)]
                for e in range(8):
                    nc.sync.dma_start(wo[e][:], woT.ap()[e * BT:(e + 1) * BT, :])
                aT = [fp.tile([BT, TPC], F32R, name=f"aT{i}") for i in range(8)]
                for i in range(8):
                    nc.sync.dma_start(aT[i][:], a2a_out[BT * i:BT * (i + 1), :])
                for tb in range(TPC // BT):
                    for oc in range(D // 512):
                        psY = psum.tile([BT, 512], F32, name="psY", tag="psqkv",
                                        bufs=2)
                        for i in range(8):
                            nc.tensor.matmul(
                                psY[:],
                                _mm(aT[i][:, tb * BT:(tb + 1) * BT]),
                                _mm(wo[i][:, oc * 512:(oc + 1) * 512]),
                                start=(i == 0), stop=(i == 7))
                        ysb = fp.tile([BT, 512], F32, name="ysb", tag="ysb",
                                      bufs=2)
                        nc.vector.tensor_add(ysb[:], psY[:],
                                             bobc[:, oc * 512:(oc + 1) * 512])
                        nc.sync.dma_start(
                            y.ap()[tb * BT:(tb + 1) * BT,
                                   oc * 512:(oc + 1) * 512],
                            ysb[:])
    nc.compile()
    return nc


_CACHE = {}


def _get_nc(plan_key, mask):
    if plan_key not in _CACHE:
        _CACHE[plan_key] = build_nc(make_plan(mask))
    return _CACHE[plan_key]


def _prep_inputs(x, mask, Wq, Wk, Wv, Wo, bo):
    xT = np.ascontiguousarray(x.reshape(TOK, D).T).astype(np.float32)
    woT = np.ascontiguousarray(Wo.T).astype(np.float32)
    bo1 = np.ascontiguousarray(bo[None, :]).astype(np.float32)
    ident = np.eye(BT, dtype=np.float32)
    plan = make_plan(mask)
    in_maps = []
    for c in range(NCORES):
        rows = slice(BT * c, BT * (c + 1))
        in_maps.append({
            "xT": xT,
            "wqT": np.ascontiguousarray(Wq[rows].T).astype(np.float32),
            "wkT": np.ascontiguousarray(Wk[rows].T).astype(np.float32),
            "wvT": np.ascontiguousarray(Wv[rows].T).astype(np.float32),
            "woT": woT,
            "bo1": bo1,
            "ident": ident,
            "pm": plan["patterns"],
        })
    return in_maps


def run(inputs, trace=False, **kw):
    x = np.asarray(inputs["x"], np.float32)
    mask = np.asarray(inputs["mask"])
    plan_key = mask.tobytes()
    nc = _get_nc(plan_key, mask)
    in_maps = _prep_inputs(x, mask, np.asarray(inputs["Wq"], np.float32),
                           np.asarray(inputs["Wk"], np.float32),
                           np.asarray(inputs["Wv"], np.float32),
                           np.asarray(inputs["Wo"], np.float32),
                           np.asarray(inputs["bo"], np.float32))
    res = run_bass_kernel_spmd(nc, in_maps, core_ids=list(range(NCORES)),
                               trace=trace, **kw)
    out = np.empty((TOK, D), np.float32)
    for c in range(NCORES):
        out[TPC * c:TPC * (c + 1)] = res.results[c]["y"]
    return out.reshape(B, N, D), res


def kernel(**inputs):
    out, _ = run(inputs, trace=False)
    return out

